# revision 7
# baseline (speedup 1.0000x reference)
"""Trainium2 Bass kernel for the vq_codebook / HDC problem.

Math (reference):
    hv      = sign(feat @ proj_w.T)                  [N=16384, D=10000], +-1 (0 -> +1)
    per_cls = segment_sum(hv, labels, K=3)           [3, D]
    updated = classify_weights + 0.5 * per_cls
    protos  = updated / max(||updated||_row, eps)
    logits  = hv @ protos.T                          [N, 3]

Strategy (8 NeuronCores):
  * Shard along D: each core owns 1250 hyper-dims, all N rows. Per-class
    sums are then fully core-local (no mid-kernel collective at all).
  * Host sorts rows by label, so segment sums become contiguous-range sums
    along the free (n) axis -- fused into the sign op via accum_out.
  * Device computes, per core:  hvT = sign(projwT_loc.T-tiles @ featT)  in
    [d, n] layout (fp32r matmul, 1 cyc/row), sign via ACT(Sign)+accum /
    DVE(is_ge*2 then -1)+accum, hv stored bf16 in SBUF (never HBM).
    Then u2 = 2*CW_loc + S_loc (bf16) and P2 = u2-tiles.T @ hvT  ->
    [3, N] partial (un-normalized 2*logits contribution of this d-slice).
  * hv SBUF residency is grouped (d-tile groups [3,3,4]); P2 partials per
    group go straight to DRAM; host sums 8 cores x 3 groups, applies the
    1/(2*norm) scale and un-permutes rows.  All heavy lifting (>99.99% of
    FLOPs and bytes) is on-device; host does only O(N*K + D*K) assembly.
"""

import os
import sys

sys.path.insert(0, "/opt/trn_rl_repo")
os.environ.setdefault("MYCRO_LOCAL_CACHE", "1")

import numpy as np

import concourse.bass as bass
import concourse.tile as tile
from concourse import bacc
from concourse import mybir
from concourse.bass import MemorySpace
from concourse.bass_utils import run_bass_kernel_spmd

# ---------------------------------------------------------------- constants
N = 16384          # rows
C = 128            # feat dim (contraction)
D = 10000          # hyper dim
K = 3              # classes
NCORES = 8
DLOC = D // NCORES          # 1250 per core
PT = 125                    # partitions per d-tile
NT = DLOC // PT             # 10 d-tiles per core
NCH = 512                   # n-chunk (matmul moving free size)
NJ = N // NCH               # 32 chunks
PCH = 1024                  # P2 psum superchunk
NJJ = N // PCH              # 16
SCH = 1024                  # sign/evac chunk (2 psum banks, 2 matmuls)
NSC = N // SCH              # 16
GROUPS = [[0, 1, 2], [3, 4, 5], [6, 7, 8, 9]]
NG = len(GROUPS)
HV_BUFS = 5                 # SBUF slots for [PT, N] bf16 hv tiles (32KB/part each)
ACT_MOD = 4                 # sign tiles: ACT engine unless idx % ACT_MOD == ACT_MOD-1
MM_DT = mybir.dt.float32r   # encode-matmul dtype (1 cyc/row vs 4 for fp32)

LAM = 0.5
EPS = 1e-12

LAST_RESULTS = None         # BassKernelResults of the most recent run (for test.py)


def _chunk_segments(j, cuts):
    """Segments of chunk [j*NCH, (j+1)*NCH) split at sorted-label boundaries.

    Returns [(s0, s1, cls)] with s0/s1 chunk-relative."""
    lo, hi = j * SCH, (j + 1) * SCH
    pts = [lo] + [b for b in cuts if lo < b < hi] + [hi]
    segs = []
    for a, b in zip(pts[:-1], pts[1:]):
        cls = 0 if a < cuts[0] else (1 if a < cuts[1] else 2)
        segs.append((a - lo, b - lo, cls))
    return segs


def build_nc(cuts):
    """Build the single-core Bass program (same for all cores; only DRAM
    inputs differ per core).  cuts = [c0, c0+c1] sorted-label boundaries."""
    nc = bacc.Bacc()
    featT = nc.dram_tensor("featT", [C, N], MM_DT, kind="ExternalInput")
    projwT = nc.dram_tensor("projwT", [C, DLOC], MM_DT, kind="ExternalInput")
    cw2t = nc.dram_tensor("cw2t", [PT, NT * K], mybir.dt.float32, kind="ExternalInput")
    p_out = nc.dram_tensor("p_out", [NG, K, N], mybir.dt.float32, kind="ExternalOutput")
    s_out = nc.dram_tensor("s_out", [PT, NT * K], mybir.dt.float32, kind="ExternalOutput")

    # per-chunk segment tables (identical for every t)
    seg_table = [_chunk_segments(j, cuts) for j in range(NSC)]
    ncols = sum(len(s) for s in seg_table)  # accum columns per d-tile

    with tile.TileContext(nc) as tc:
        with (
            tc.tile_pool(name="singles", bufs=1) as singles,
            tc.tile_pool(name="hv", bufs=HV_BUFS) as hvp,
            tc.tile_pool(name="feat", bufs=3) as featp,
            tc.tile_pool(name="spart", bufs=6) as spartp,
            tc.tile_pool(name="pstage", bufs=2) as pstp,
            tc.tile_pool(name="mm1ps", bufs=2, space=MemorySpace.PSUM) as mm1ps,
            tc.tile_pool(name="pps", bufs=2, space=MemorySpace.PSUM) as pps,
        ):
            projw_sb = singles.tile([C, DLOC], MM_DT)
            nc.sync.dma_start(out=projw_sb, in_=projwT[:, :])
            cw2_sb = singles.tile([PT, NT * K], mybir.dt.float32)
            nc.sync.dma_start(out=cw2_sb, in_=cw2t[:, :])
            upd_sb = singles.tile([PT, NT * K], mybir.dt.bfloat16)
            s_sb = singles.tile([PT, NT * K], mybir.dt.float32)
            nc.vector.memset(s_sb, 0.0)

            sidx = 0  # sign-tile counter for ACT/DVE balance
            for g, ts in enumerate(GROUPS):
                hv = {}
                spart = {}
                for t in ts:
                    hv[t] = hvp.tile([PT, N], mybir.dt.bfloat16, tag="hv", name=f"hv{t}")
                    spart[t] = spartp.tile([PT, 40], mybir.dt.float32, tag="sp", name=f"sp{t}")

                # ---- produce: hvT tiles + per-segment sums --------------
                # 1024-wide sign chunks (2 psum banks, 2 matmuls each) halve
                # ACT instruction count + accum-readout + sem overhead.
                for j in range(NSC):
                    fj = featp.tile([C, SCH], MM_DT, tag="fj")
                    nc.sync.dma_start(out=fj, in_=featT[:, j * SCH:(j + 1) * SCH])
                    for t in ts:
                        ps = mm1ps.tile([PT, SCH], mybir.dt.float32, tag="mm1")
                        for h in range(SCH // NCH):
                            nc.tensor.matmul(
                                ps[:, h * NCH:(h + 1) * NCH],
                                projw_sb[:, t * PT:(t + 1) * PT],
                                fj[:, h * NCH:(h + 1) * NCH],
                                start=True, stop=True,
                            )
                        col0 = sum(len(seg_table[jj]) for jj in range(j))
                        on_act = (sidx % ACT_MOD) != (ACT_MOD - 1)
                        sidx += 1
                        for si, (s0, s1, _cls) in enumerate(seg_table[j]):
                            hv_sl = hv[t][:, j * SCH + s0: j * SCH + s1]
                            acc = spart[t][:, col0 + si: col0 + si + 1]
                            if on_act:
                                nc.scalar.activation(
                                    hv_sl, ps[:, s0:s1],
                                    mybir.ActivationFunctionType.Sign,
                                    accum_out=acc,
                                )
                            else:
                                nc.vector.tensor_scalar(
                                    hv_sl, ps[:, s0:s1], 0.0, 2.0,
                                    mybir.AluOpType.is_ge, mybir.AluOpType.mult,
                                )
                                # op1 is the accum reduction op (add), not elementwise
                                nc.vector.tensor_scalar(
                                    hv_sl, hv_sl, -1.0, None,
                                    mybir.AluOpType.add, mybir.AluOpType.add,
                                    accum_out=acc,
                                )

                # ---- collapse segment partials -> S, u2 -----------------
                # accum columns are in non-decreasing class order
                col_cls = [cls for j in range(NSC) for (_a, _b, cls) in seg_table[j]]
                for t in ts:
                    for k in range(K):
                        idxs = [i for i, cc in enumerate(col_cls) if cc == k]
                        if not idxs:
                            continue
                        a, b = idxs[0], idxs[-1] + 1
                        nc.vector.reduce_sum(
                            s_sb[:, t * K + k: t * K + k + 1],
                            spart[t][:, a:b],
                            axis=mybir.AxisListType.X,
                        )
                    nc.vector.tensor_add(
                        upd_sb[:, t * K:(t + 1) * K],
                        s_sb[:, t * K:(t + 1) * K],
                        cw2_sb[:, t * K:(t + 1) * K],
                    )

                # ---- consume: P2 partial = u2.T @ hvT -------------------
                for jj in range(NJJ):
                    pp = pps.tile([K, PCH], mybir.dt.float32, tag="pp")
                    for i, t in enumerate(ts):
                        for h in range(2):
                            nc.tensor.matmul(
                                pp[:, h * NCH:(h + 1) * NCH],
                                upd_sb[:, t * K:(t + 1) * K],
                                hv[t][:, jj * PCH + h * NCH: jj * PCH + (h + 1) * NCH],
                                start=(i == 0), stop=(i == len(ts) - 1),
                            )
                    pst = pstp.tile([K, PCH], mybir.dt.float32, tag="pst")
                    nc.vector.tensor_copy(pst, pp)
                    nc.sync.dma_start(
                        out=p_out[g, :, jj * PCH:(jj + 1) * PCH], in_=pst
                    )

            nc.sync.dma_start(out=s_out[:, :], in_=s_sb)
    nc.compile()
    return nc


def _prep_inputs(feat_s, proj_w, classify_weights):
    featT = np.ascontiguousarray(feat_s.T).astype(np.float32)  # [128, N]
    in_maps = []
    for core in range(NCORES):
        sl = slice(core * DLOC, (core + 1) * DLOC)
        projwT = np.ascontiguousarray(proj_w[sl].T).astype(np.float32)  # [128, DLOC]
        cw2 = (2.0 * classify_weights[:, sl].astype(np.float32)).T      # [DLOC, 3]
        cw2t = np.ascontiguousarray(
            cw2.reshape(NT, PT, K).transpose(1, 0, 2).reshape(PT, NT * K)
        )
        in_maps.append({"featT": featT, "projwT": projwT, "cw2t": cw2t})
    return in_maps


def kernel(feat, proj_w, classify_weights, labels, _trace=False):
    global LAST_RESULTS
    feat = np.asarray(feat, dtype=np.float32)
    proj_w = np.asarray(proj_w, dtype=np.float32)
    classify_weights = np.asarray(classify_weights, dtype=np.float32)
    labels = np.asarray(labels).astype(np.int64)

    perm = np.argsort(labels, kind="stable")
    feat_s = feat[perm]
    counts = np.bincount(labels, minlength=K)
    cuts = [int(counts[0]), int(counts[0] + counts[1])]

    nc = build_nc(cuts)
    in_maps = _prep_inputs(feat_s, proj_w, classify_weights)
    res = run_bass_kernel_spmd(
        nc, in_maps, list(range(NCORES)), trace=_trace
    )
    LAST_RESULTS = res

    S = np.zeros((K, D), np.float32)
    P2 = np.zeros((K, N), np.float64)
    for core in range(NCORES):
        s_o = np.asarray(res.results[core]["s_out"])          # [PT, NT*K]
        s_full = s_o.reshape(PT, NT, K).transpose(1, 0, 2).reshape(DLOC, K)
        S[:, core * DLOC:(core + 1) * DLOC] = s_full.T
        P2 += np.asarray(res.results[core]["p_out"]).astype(np.float64).sum(axis=0)

    updated = classify_weights + np.float32(LAM) * S          # [K, D] f32
    norms = np.linalg.norm(updated, axis=1)                   # f32-ish norms
    scale = 0.5 / np.maximum(norms, EPS)
    logits_sorted = (P2 * scale[:, None]).T.astype(np.float32)  # [N, K]
    out = np.empty((N, K), np.float32)
    out[perm] = logits_sorted
    return out
